# revision 80
# baseline (speedup 1.0000x reference)
"""Trainium2 Bass kernel for a GPT-2 style transformer block, 8-core SPMD.

Sharding: core c handles sequence c//2, query-parity c%2. Each core owns the
8 interleaved 128-token query blocks (even core: blocks 0,2,..,14; odd core:
blocks 1,3,..,15) of its sequence, computes full k/v for that sequence, runs
causal attention for its query blocks (all 16 heads), then the MLP for its
1024 tokens. Outputs are scattered back on the host. No collectives; the
per-core programs are instruction-identical (SPMD) — causality differences
are encoded in per-core mask *data* (multiplicative {0,1} masks on the last
two key-tiles of each query block).

Attention layout: scores are computed transposed (sT[key, q] = k . q) so that
both PV operands are keys-on-partition and softmax needs no transposes; the
denominator comes from an appended ones-column in V (out = [av | sum(p)]),
and the per-row normalization happens token-major where it is a cheap
per-partition scalar multiply.

Precision: qkv + aproj run fp8-e4m3 DoubleRow (weights pre-scaled x64,
undone at PSUM readout; 2 contraction rows per partition -> 256-deep
matmuls); attention core and the MLP stay bf16 (fp8 MLP exceeds the error
budget). Residual/layernorm fp32.

Scheduling: attention, aproj and LN1 are fused in one g-outer/head-inner
phase; PV is staggered one head behind scores so the PE streams while
ScalarE runs exp. The late k/v projection halves and most of the fc
matmuls are interleaved into the attention head slots — this feeds the PE
fat N=512 matmuls throughout, which keeps the HAM clock-gate at 2.4 GHz
(the small score/PV matmuls alone leave it throttled at 1.2 GHz) and
overlaps MLP work with the exp-bound softmax. fc's gelu is deferred and
batched so ScalarE never thrashes activation-table loads mid-phase.
"""

import sys
from contextlib import ExitStack

import numpy as np

for _p in ("/opt/trn_rl_repo",):
    if _p not in sys.path:
        sys.path.insert(0, _p)

import ml_dtypes

import concourse.bass as bass
import concourse.mybir as mybir
import concourse.tile as tile
from concourse import bacc
from concourse.bass_utils import run_bass_kernel_spmd
from concourse.masks import make_identity

BF16 = ml_dtypes.bfloat16
E4M3 = ml_dtypes.float8_e4m3
F32 = mybir.dt.float32
BF = mybir.dt.bfloat16
F8 = mybir.dt.float8e4
P = 128
AF = mybir.ActivationFunctionType
ALU = mybir.AluOpType
DR = mybir.MatmulPerfMode.DoubleRow
WSCL = 64.0  # fp8 weight pre-scale (std 0.02 -> 1.28), undone at PSUM readout


def chunks(total, size=512):
    return [(s, min(size, total - s)) for s in range(0, total, size)]


class Cfg:
    """Problem geometry. Defaults = the real problem; overridable for sims."""

    def __init__(self, S=2048, D=1024, H=16, DFF=4096, ln2_trivial=False,
                 ln1_trivial=False):
        self.ln2_trivial = ln2_trivial
        self.ln1_trivial = ln1_trivial
        self.S = S
        self.D = D
        self.H = H
        self.DFF = DFF
        self.HD = 64
        assert self.H * self.HD == self.D
        self.NQ = S // 2
        self.NSLOT = S // 256
        self.KT = D // P
        self.DK = D // (2 * P)
        self.MT_FF = DFF // P
        self.NQ_T = self.NQ // P
        self.S_T = S // P


def build_nc(cfg: Cfg):
    S, D, H, DFF = cfg.S, cfg.D, cfg.H, cfg.DFF
    NQ, NSLOT, KT, MT_FF = cfg.NQ, cfg.NSLOT, cfg.KT, cfg.MT_FF
    NQ_T, S_T, DK = cfg.NQ_T, cfg.S_T, cfg.DK
    NPAIR = H // 2

    nc = bacc.Bacc(None, target_bir_lowering=False, debug=False)

    def din(name, shape, d=BF):
        return nc.dram_tensor(name, shape, d, kind="ExternalInput").ap()

    # fp8 DoubleRow operands: contraction row 256*dk + 128*j + p lives at
    # [p, dk, j] so a [P, 2, cols] slice contracts 256 rows per matmul.
    xt = din("xt", [P, DK, 2, S], F8)      # x[b].T packed for DoubleRow
    xtq = din("xtq", [P, DK, 2, NQ], F8)   # own tokens, same packing
    xq = din("xq", [NQ, D], F32)           # own tokens, token-major (residual)
    masks = din("masks", [4, P, 2 * P])    # multiplicative causal masks (bf16)
    wqk = din("wqk", [P, 2 * KT, DK, 2, P], F8)   # q,k col-tiles (x WSCL)
    wv = din("wv", [P, 2, DK, 2, 512], F8)        # v col-chunks (x WSCL)
    bqkv = din("bqkv", [P, 2 * KT], F32)   # q,k bias packed (col%128, col//128)
    bvrow = din("bvrow", [1, D])           # v bias as a row (bf16)
    wap = din("wap", [P, DK, 2, D], F8)    # aproj rows (x WSCL)
    wfc = din("wfc", [P, MT_FF, KT, P])
    bfc = din("bfc", [P, MT_FF], F32)
    wmp = din("wmp", [DFF, D])
    bmp = din("bmp", [1, D], F32)
    g1 = din("g1", [1, D], F32)
    g2 = din("g2", [1, D], F32)
    b2 = din("b2", [1, D], F32)
    # bf16 output halves the writeback (the final tail); host upcasts.
    # Costs ~0.11% rms on top of 2.8e-3 total err vs the 2e-2 gate.
    out = nc.dram_tensor("out", [NQ, D], BF, kind="ExternalOutput").ap()

    wmp3 = wmp.rearrange("(kt p) c -> kt p c", p=P)

    def bcast_row(ap):  # [1, D] DRAM -> [P, D] broadcast AP
        return bass.AP(tensor=ap.tensor, offset=ap.offset, ap=[[0, P], [1, D]])

    NSUB = (D + 511) // 512
    SUB = D // NSUB
    assert SUB * NSUB == D and SUB <= 512

    def layer_norm(nc, pool, dst, src, g_row, b_row, eps_t, tag, rows=None):
        """dst[P, D] = g * (src - mean)/sqrt(var+eps) + b, rowwise over D.
        rows=False skips the g/b application entirely."""
        skip_rows = rows is False
        rows = nc.gpsimd if rows in (None, False) else rows
        stats = pool.tile([P, NSUB, 6], F32, tag=f"{tag}_st")
        for sub in range(NSUB):
            nc.vector.bn_stats(stats[:, sub, :], src[:, sub * SUB:(sub + 1) * SUB])
        mv = pool.tile([P, 2], F32, tag=f"{tag}_mv")
        nc.vector.bn_aggr(mv, stats)
        rstd = pool.tile([P, 1], F32, tag=f"{tag}_rs")
        nc.scalar.activation(out=rstd, in_=mv[:, 1:2], func=AF.Sqrt,
                             bias=eps_t, scale=1.0)
        nc.vector.reciprocal(rstd, rstd)
        nc.vector.tensor_scalar(
            out=dst, in0=src, scalar1=mv[:, 0:1], scalar2=rstd,
            op0=ALU.subtract, op1=ALU.mult)
        if not skip_rows:
            rows.tensor_mul(dst, dst, g_row)
            rows.tensor_add(dst, dst, b_row)

    with tile.TileContext(nc) as tc, ExitStack() as top:
        const = top.enter_context(tc.tile_pool(name="const", bufs=1))
        bqkvS = const.tile([P, 2 * KT], F32)
        nc.sync.dma_start(out=bqkvS, in_=bqkv)
        epsS = const.tile([P, 1], F32)
        nc.vector.memset(epsS, 1e-5)
        bvS = const.tile([P, D], BF)  # dma issued later, off the q critical path

        npool = top.enter_context(tc.tile_pool(name="npool", bufs=1))
        n_sb = [npool.tile([P, D], BF, name=f"n{mt}", tag=f"n{mt}")
                for mt in range(NQ_T)]
        NHALF = max(1, NQ // 512)
        nT = [[npool.tile([P, min(512, NQ)], BF, name=f"nT{ck}_{hf}",
                          tag=f"nT{ck}_{hf}") for hf in range(NHALF)]
              for ck in range(KT)]
        # fc token-chunk 0 is interleaved into the attention phase (keeps the
        # PE HAM-warm through the small-matmul stream); gelu is deferred to
        # phase D so ScalarE never switches tables away from Exp mid-phase.
        fc_inter = (MT_FF == 2 * H and NQ >= 512 and NHALF == 2)
        NFC_I = MT_FF - 5 if fc_inter else 0  # m-tiles run inside attention
        if fc_inter:
            preact = npool.tile([P, NFC_I, 512], BF)
            bfcSe = npool.tile([P, MT_FF], F32)

        with ExitStack() as s_a:
            preh = s_a.enter_context(tc.tile_pool(name="pre_xt", bufs=1))
            aper = s_a.enter_context(tc.tile_pool(name="aper", bufs=1))
            maskS = aper.tile([P, 4, 2 * P], BF)

            with ExitStack() as s_qkv:
                qkvp = s_qkv.enter_context(tc.tile_pool(name="qkvper", bufs=1))
                qT = qkvp.tile([P, NPAIR, NQ], BF)
                kT = qkvp.tile([P, NPAIR, S], BF)
                vA = qkvp.tile([P, S_T, H, 65], BF)

                # ---------------- Phase A: qkv projections (fp8) ----------
                # Startup transfers: each trigger engine's DMAs serialize on
                # its queue, so the q critical path (xtq + wq) issues FIRST
                # on all three queues; bulk/late loads (xtS, masks, biases)
                # queue up behind them.
                xtS = preh.tile([P, DK, 2, S], F8, tag="bigslot")

                # A1: q projection (own tokens)
                with ExitStack() as ph:
                    pa = ph.enter_context(tc.tile_pool(name="pa1", bufs=1))
                    wp = ph.enter_context(tc.tile_pool(name="pa1_w", bufs=3))
                    pp = ph.enter_context(
                        tc.tile_pool(name="pa1_ps", bufs=4, space="PSUM"))
                    xtqS = pa.tile([P, DK, 2, NQ], F8)
                    # first token-half split by dk across two queues so the
                    # first q matmuls unblock in ~half the transfer time
                    nc.sync.dma_start(out=xtqS[:, 0:DK // 2, :, 0:NQ // 2],
                                      in_=xtq[:, 0:DK // 2, :, 0:NQ // 2])
                    nc.gpsimd.dma_start(out=xtqS[:, DK // 2:, :, 0:NQ // 2],
                                        in_=xtq[:, DK // 2:, :, 0:NQ // 2])
                    wqa = wp.tile([P, NPAIR, DK, 2, P], F8, tag="wq", bufs=1)
                    nc.scalar.dma_start(out=wqa[:, 0:NPAIR // 2],
                                        in_=wqk[:, 0:NPAIR // 2])
                    nc.gpsimd.dma_start(out=wqa[:, NPAIR // 2:NPAIR],
                                        in_=wqk[:, NPAIR // 2:NPAIR])
                    nc.sync.dma_start(out=xtqS[:, :, :, NQ // 2:],
                                      in_=xtq[:, :, :, NQ // 2:])
                    # bulk loads, behind the q-path on each queue
                    dma_engs = [nc.gpsimd, nc.scalar, nc.gpsimd, nc.sync]
                    for i, (h0, hw) in enumerate(chunks(S, S // 4)):
                        dma_engs[i].dma_start(
                            out=xtS[:, :, :, h0:h0 + hw],
                            in_=xt[:, :, :, h0:h0 + hw])
                    nc.sync.dma_start(
                        out=maskS, in_=masks.rearrange("m k q -> k m q"))
                    nc.scalar.dma_start(out=bvS, in_=bcast_row(bvrow))
                    if fc_inter:
                        nc.sync.dma_start(out=bfcSe, in_=bfc)
                    for c0, w in chunks(NQ):
                        for hp in range(NPAIR):
                            ps = pp.tile([P, 512], F32, tag="ps")
                            for dk in range(DK):
                                nc.tensor.matmul(
                                    ps[:, :w], lhsT=wqa[:, hp, dk],
                                    rhs=xtqS[:, dk, :, c0:c0 + w],
                                    start=(dk == 0), stop=(dk == DK - 1),
                                    perf_mode=DR)
                            nc.scalar.activation(
                                out=qT[:, hp, c0:c0 + w], in_=ps[:, :w],
                                func=AF.Identity, bias=bqkvS[:, hp:hp + 1],
                                scale=1.0 / (8.0 * WSCL))

                # A2: k/v projections. The late halves (k columns >= S/2,
                # v token-tiles >= S_T/2) are deferred into the g0/g1
                # attention slots: keeps the PE HAM-warm there and shortens
                # the serial phase A. Consumers only need them from g2 on.
                wvS = qkvp.tile([P, 2, DK, 2, 512], F8)
                nc.gpsimd.dma_start(out=wvS, in_=wv)

                def k_unit(wk, hp, c0, pstile, eng):
                    w = 512
                    for dk in range(DK):
                        nc.tensor.matmul(
                            pstile[:, :w], lhsT=wk[:, dk],
                            rhs=xtS[:, dk, :, c0:c0 + w],
                            start=(dk == 0), stop=(dk == DK - 1),
                            perf_mode=DR)
                    eng.tensor_scalar(
                        out=kT[:, hp, c0:c0 + w], in0=pstile[:, :w],
                        scalar1=1.0 / WSCL,
                        scalar2=bqkvS[:, KT + hp:KT + hp + 1],
                        op0=ALU.mult, op1=ALU.add)

                def v_unit(ci, mt, pstile):
                    c0, cw = ci * 512, 512
                    h0, nh = c0 // 64, cw // 64
                    for dk in range(DK):
                        nc.tensor.matmul(
                            pstile[:, :cw],
                            lhsT=xtS[:, dk, :, mt * P:(mt + 1) * P],
                            rhs=wvS[:, ci, dk],
                            start=(dk == 0), stop=(dk == DK - 1),
                            perf_mode=DR)
                    nc.vector.scalar_tensor_tensor(
                        out=vA[:, mt, h0:h0 + nh, 0:64],
                        in0=pstile[:, :cw]
                        .rearrange("p (h d) -> p h d", d=64),
                        scalar=1.0 / WSCL,
                        in1=bvS[:, c0:c0 + cw]
                        .rearrange("p (h d) -> p h d", d=64),
                        op0=ALU.mult, op1=ALU.add)

                kv_defer = (S == 2048 and NPAIR == 8 and S_T == 16)
                with ExitStack() as ph:
                    wp = ph.enter_context(tc.tile_pool(name="pa2_w", bufs=3))
                    pp = ph.enter_context(
                        tc.tile_pool(name="pa2_ps", bufs=4, space="PSUM"))
                    kc = [(hp, c0) for hp in range(NPAIR)
                          for c0, _ in chunks(S)]
                    if kv_defer:
                        kc = [(hp, c0) for hp, c0 in kc if c0 < S // 2]
                    lasthp = None
                    for hp, c0 in kc:
                        if hp != lasthp:
                            wk = wp.tile([P, DK, 2, P], F8, tag="wk")
                            nc.scalar.dma_start(out=wk, in_=wqk[:, KT + hp])
                            lasthp = hp
                        k_unit(wk, hp, c0,
                               pp.tile([P, 512], F32, tag="ps", name="ps"),
                               nc.vector)
                    vc = [(ci, mt) for ci in range(2) for mt in range(S_T)]
                    if kv_defer:
                        vc = [(ci, mt) for ci, mt in vc if mt < S_T // 2]
                    for ci, mt in vc:
                        v_unit(ci, mt,
                               pp.tile([P, 512], F32, tag="ps", name="ps"))
                    nc.vector.memset(vA[:, :, :, 64:65], 1.0)

                # deferred-unit issue list for the g0/g1 attention slots:
                # k late chunks first (needed at g2), then v late tiles
                # v first: wvS is already resident (no per-unit weight DMA),
                # so g0 gets fat matmuls immediately; k units at g1 give the
                # weight prefetch a full group of slack. k needed from g2 on.
                kv_units = []
                if kv_defer:
                    for mt in range(S_T // 2, S_T):
                        for ci in range(2):
                            kv_units.append(("v", ci, mt))
                    for hp in range(NPAIR):
                        for c0 in (S // 2, S // 2 + 512):
                            kv_units.append(("k", hp, c0))

                # wap for the per-group aproj inside the fused phase
                wapS = preh.tile([P, DK, 2, D], F8, tag="wapslot",
                                 name="wapS")
                nc.gpsimd.dma_start(out=wapS, in_=wap)

                # ------- Phase B+C fused: attention + aproj + LN1 -------
                # g-outer/head-inner; PV staggered one head behind scores so
                # the PE streams scores(h+1) while ScalarE exps head h; aproj
                # + LN1 run per 256-token group to keep the PE warm (HAM).
                with ExitStack() as ph:
                    psp = ph.enter_context(
                        tc.tile_pool(name="pb_s", bufs=2, space="PSUM"))
                    pvp = ph.enter_context(
                        tc.tile_pool(name="pb_av", bufs=1, space="PSUM"))
                    paux = ph.enter_context(
                        tc.tile_pool(name="pb_aux", bufs=2, space="PSUM"))
                    ptp = ph.enter_context(tc.tile_pool(name="pb_pt", bufs=4))
                    pm = ph.enter_context(tc.tile_pool(name="pb_m", bufs=4))
                    pc = ph.enter_context(tc.tile_pool(name="pb_c", bufs=1))
                    pwf = ph.enter_context(tc.tile_pool(name="pb_wf", bufs=2))

                    idb = pc.tile([P, P], BF)
                    make_identity(nc, idb)

                    mt_per_half = NQ_T // NHALF

                    def head_scores(g, h, nkt, ch):
                        # ch: k-tiles per score chunk. Finer chunks in the
                        # early groups pipeline exp against the PE tighter
                        # (the PE waits on exp to free the PSUM ring).
                        hp, hh = h // 2, h % 2
                        pb = hh * 64
                        pts = []
                        for sc in range(nkt // ch):
                            lo = sc * ch
                            ps = psp.tile([P, ch, 2 * P], F32, tag="ps",
                                          padded_shape=[P, 4, 2 * P])
                            for kt in range(lo, lo + ch):
                                nc.tensor.matmul(
                                    ps[:, kt - lo, :],
                                    lhsT=kT[pb:pb + 64, hp,
                                            kt * P:(kt + 1) * P],
                                    rhs=qT[pb:pb + 64, hp,
                                           g * 2 * P:(g + 1) * 2 * P],
                                    start=True, stop=True)
                            pt = ptp.tile([P, ch, 2 * P], BF,
                                          tag=f"pt{h % 2}", name="pt",
                                          padded_shape=[P, 4, 2 * P])
                            nc.scalar.activation(out=pt, in_=ps, func=AF.Exp)
                            if lo >= nkt - 4:  # masks cover last 4 k-tiles
                                mo = lo - (nkt - 4)
                                nc.vector.tensor_mul(
                                    pt, pt, maskS[:, mo:mo + ch])
                            pts.append(pt)
                        return pts

                    def head_pv(g, h, nkt, ch, pts, aL, aR):
                        pavL = pvp.tile([P, 65], F32, tag="pavL")
                        pavR = pvp.tile([P, 65], F32, tag="pavR")
                        for kt in range(nkt - 2):
                            nc.tensor.matmul(
                                pavL, lhsT=pts[kt // ch][:, kt % ch, 0:P],
                                rhs=vA[:, kt, h, :],
                                start=(kt == 0), stop=(kt == nkt - 3))
                        for kt in range(nkt):
                            nc.tensor.matmul(
                                pavR, lhsT=pts[kt // ch][:, kt % ch,
                                                         P:2 * P],
                                rhs=vA[:, kt, h, :],
                                start=(kt == 0), stop=(kt == nkt - 1))
                        for pav, aB in ((pavL, aL), (pavR, aR)):
                            rec = pm.tile([P, 1], F32, tag="rec")
                            nc.vector.reciprocal(rec, pav[:, 64:65])
                            nc.vector.tensor_scalar_mul(
                                out=aB[:, h * 64:(h + 1) * 64],
                                in0=pav[:, 0:64], scalar1=rec)

                    def ntr(mt):  # n transpose -> nT (fc operand)
                        hf, mo = mt // mt_per_half, mt % mt_per_half
                        pstf = paux.tile([P, KT, P], BF, tag="paux",
                                         name="pstf")
                        for ck in range(KT):
                            nc.tensor.transpose(
                                pstf[:, ck], n_sb[mt][:, ck * P:(ck + 1) * P],
                                idb)
                        for ck in range(KT):
                            nc.vector.tensor_scalar_mul(
                                out=nT[ck][hf][:, mo * P:(mo + 1) * P],
                                in0=pstf[:, ck], scalar1=1.0)

                    kv_wk = {}

                    def kv_prefetch(ui):
                        # prefetch the k-pair weight for units [ui, ui+1]
                        if ui % 2 or ui >= len(kv_units) or ui in kv_wk:
                            return
                        kind, a, _ = kv_units[ui]
                        if kind != "k":
                            return
                        wkd = pwf.tile([P, DK, 2, P], F8, tag="wfc",
                                       name="wkd")
                        (nc.gpsimd if ui % 4 else nc.sync).dma_start(
                            out=wkd, in_=wqk[:, KT + a])
                        kv_wk[ui] = wkd

                    def kv_slot(ui):
                        kind, a, b = kv_units[ui]
                        pstile = paux.tile([P, 512], F32, tag="paux",
                                           name="kvps")
                        if kind == "k":
                            base = ui - (ui % 2)
                            kv_prefetch(base)       # no-op if prefetched
                            kv_prefetch(base + 2)   # hide next pair's DMA
                            k_unit(kv_wk[base], a, b, pstile, nc.vector)
                            if ui % 2 == 1:
                                kv_wk.pop(base)
                        else:
                            v_unit(a, b, pstile)
                            if ui + 2 < len(kv_units) \
                                    and kv_units[ui + 2][0] == "k":
                                kv_prefetch(ui + 2)

                    fc_w = {}

                    def fc_prefetch(m):
                        if m >= NFC_I or m in fc_w:
                            return
                        wfcT = pwf.tile([P, KT, P], BF, tag="wfc",
                                        name="wfcT")
                        (nc.gpsimd if m % 2 else nc.sync).dma_start(
                            out=wfcT, in_=wfc[:, m])
                        fc_w[m] = wfcT

                    def fc_mtile(m):
                        # weight was prefetched one slot ahead — the DMA
                        # latency is hidden behind the previous slot's work
                        wfcT = fc_w.pop(m)
                        fcps = paux.tile([P, 512], F32, tag="paux",
                                         name="fcps")
                        for kt in range(KT):
                            nc.tensor.matmul(
                                fcps, lhsT=wfcT[:, kt, :],
                                rhs=nT[kt][0][:, 0:512],
                                start=(kt == 0), stop=(kt == KT - 1))
                        nc.vector.tensor_scalar_add(
                            out=preact[:, m], in0=fcps,
                            scalar1=bfcSe[:, m:m + 1])

                    def block_post(j, aB, jprev):
                        # a^T (fp8 DoubleRow operand) via PE transpose
                        pst = paux.tile([P, KT, P], BF, tag="paux",
                                        name="pst")
                        for ck in range(KT):
                            nc.tensor.transpose(
                                pst[:, ck], aB[:, ck * P:(ck + 1) * P], idb)
                        aTb = [pc.tile([P, 2, P], F8, name=f"aT{dk}",
                                       tag=f"aT{dk}", bufs=1)
                               for dk in range(DK)]
                        for ck in range(KT):
                            nc.vector.tensor_scalar_mul(
                                out=aTb[ck // 2][:, ck % 2, :],
                                in0=pst[:, ck], scalar1=1.0)
                        xqS = pm.tile([P, D], F32, tag="xqS", bufs=1)
                        nc.sync.dma_start(out=xqS,
                                          in_=xq[j * P:(j + 1) * P, :])
                        for c0, w in chunks(D):
                            aps = paux.tile([P, 512], F32, tag="paux",
                                            name="aps")
                            for dk in range(DK):
                                nc.tensor.matmul(
                                    aps[:, :w],
                                    lhsT=aTb[dk],
                                    rhs=wapS[:, dk, :, c0:c0 + w],
                                    start=(dk == 0), stop=(dk == DK - 1),
                                    perf_mode=DR)
                            sl = slice(c0, c0 + w)
                            nc.vector.scalar_tensor_tensor(
                                out=xqS[:, sl], in0=aps[:, :w],
                                scalar=1.0 / WSCL,
                                in1=xqS[:, sl], op0=ALU.mult, op1=ALU.add)
                        layer_norm(nc, pm, n_sb[j], xqS, None, None,
                                   epsS, "ln1", rows=False)
                        if jprev is not None:
                            ntr(jprev)  # PE work overlapping LN1(j) on DVE

                    prev = None
                    jprev = None
                    NG = NSLOT // 2
                    for g in range(NG):
                        nkt = 4 * g + 4
                        jL, jR = 2 * g, 2 * g + 1
                        aL = aper.tile([P, D], BF, tag="aSB", bufs=3,
                                       name=f"aSB{jL}")
                        aR = aper.tile([P, D], BF, tag="aSB", bufs=3,
                                       name=f"aSB{jR}")
                        for h in range(H):
                            ch = 2 if nkt <= 8 else 4
                            pts = head_scores(g, h, nkt, ch)
                            if prev is not None:
                                head_pv(*prev)
                            prev = (g, h, nkt, ch, pts, aL, aR)
                            if fc_inter and g >= NG - 2:
                                mi = (g - (NG - 2)) * H + h
                                fc_prefetch(mi + 1)
                                if mi < NFC_I:
                                    fc_mtile(mi)
                            elif kv_units and g < 2:
                                ui = g * H + h
                                if ui < len(kv_units):
                                    kv_slot(ui)
                        head_pv(*prev)
                        prev = None
                        block_post(jL, aL, jprev)
                        block_post(jR, aR, jL)
                        jprev = jR
                        if fc_inter and g == NG - 3:
                            # flush: g+1 interleaves fc reads of nT[..][0],
                            # which needs block jR's transpose done now
                            ntr(jR)
                            jprev = None
                            fc_prefetch(0)  # first fc weight, one g early
                    ntr(jprev)
            # qT/kT/vA freed here

            # residual copy: n = g1*n0 + (b1 + b_mproj), off critical path
            if not cfg.ln1_trivial:
                g1S = npool.tile([P, D], F32)
                nc.sync.dma_start(out=g1S, in_=bcast_row(g1))
                bmpS = npool.tile([P, D], F32)
                nc.sync.dma_start(out=bmpS, in_=bcast_row(bmp))
                for mt in range(NQ_T):
                    nc.gpsimd.tensor_mul(n_sb[mt], n_sb[mt], g1S)
                    nc.gpsimd.tensor_add(n_sb[mt], n_sb[mt], bmpS)
        # aSB freed here

        # -------- Phase D: MLP + residual + LN2, in half-token chunks --------
        with ExitStack() as ph:
            pd = ph.enter_context(tc.tile_pool(name="pd", bufs=1))
            pdw = ph.enter_context(tc.tile_pool(name="pd_w", bufs=8))
            pmd = ph.enter_context(tc.tile_pool(name="pd_m", bufs=2))
            ppd = ph.enter_context(
                tc.tile_pool(name="pd_ps", bufs=2, space="PSUM"))
            ppm = ph.enter_context(
                tc.tile_pool(name="pd_psm", bufs=1, space="PSUM"))

            bfcS = pd.tile([P, MT_FF], F32)
            nc.sync.dma_start(out=bfcS, in_=bfc)
            g2S = pd.tile([P, D], F32)
            nc.sync.dma_start(out=g2S, in_=bcast_row(g2))
            b2S = pd.tile([P, D], F32)
            nc.sync.dma_start(out=b2S, in_=bcast_row(b2))

            TC = min(512, NQ)          # token-chunk

            def fc_mtile_d(m, hf, hTc):
                wfcT = pdw.tile([P, KT, P], BF, tag="wfc")
                nc.scalar.dma_start(out=wfcT, in_=wfc[:, m])
                ps = ppd.tile([P, 512], F32, tag="ps")
                for kt in range(KT):
                    nc.tensor.matmul(
                        ps, lhsT=wfcT[:, kt, :], rhs=nT[kt][hf][:, 0:TC],
                        start=(kt == 0), stop=(kt == KT - 1))
                nc.scalar.activation(
                    out=hTc[:, m, :], in_=ps,
                    func=AF.Gelu_apprx_tanh, bias=bfcS[:, m:m + 1],
                    scale=1.0)

            r2s = {}
            st2 = {}

            def mproj_chunk(t0, hTc, split_stats=False):
                # generator: yields after each kt unit so fc work for the
                # next chunk can interleave into the mproj weight stream.
                # split_stats: issue each row-block's first-half LN2 stats
                # right after the ci=0 residual, shortening the final tail.
                mts = list(range(t0 // P, (t0 + TC) // P))
                for mt in mts:
                    r2s[mt] = pmd.tile([P, D], F32, tag=f"r2_{mt % 4}",
                                       name=f"r2_{mt}")
                for ci, (c0, w) in enumerate(chunks(D)):
                    pss = {mt: ppm.tile([P, 512], F32, tag=f"mp{mt % 4}",
                                        name=f"mp_{mt}_{ci}")
                           for mt in mts}
                    for kt in range(MT_FF):
                        wmT = pdw.tile([P, 512], BF, tag="wm")
                        nc.sync.dma_start(out=wmT[:, :w],
                                          in_=wmp3[kt][:, c0:c0 + w])
                        for mt in mts:
                            nc.tensor.matmul(
                                pss[mt][:, :w],
                                lhsT=hTc[:, kt, mt * P - t0:(mt + 1) * P - t0],
                                rhs=wmT[:, :w],
                                start=(kt == 0), stop=(kt == MT_FF - 1))
                        yield
                    sl = slice(c0, c0 + w)
                    for mt in mts:
                        nc.vector.scalar_tensor_tensor(
                            out=r2s[mt][:, sl], in0=pss[mt][:, :w],
                            scalar=1.0, in1=n_sb[mt][:, sl],
                            op0=ALU.mult, op1=ALU.add)
                        if split_stats and NSUB == 2:
                            if ci == 0:
                                st2[mt] = pmd.tile([P, NSUB, 6], F32,
                                                   tag=f"st2_{mt % 4}",
                                                   name=f"st2_{mt}")
                            nc.vector.bn_stats(st2[mt][:, ci, :],
                                               r2s[mt][:, sl])

            def ln2_out(mts, tail=False):
                for i, mt in enumerate(mts):
                    oS = pmd.tile([P, D], BF, tag="oS")
                    if tail and mt in st2:
                        # stats were issued inside mproj_chunk; finish only
                        # aggr + rstd + normalize here
                        mv = pmd.tile([P, 2], F32, tag="ln2f_mv")
                        nc.vector.bn_aggr(mv, st2[mt])
                        rstd = pmd.tile([P, 1], F32, tag="ln2f_rs")
                        nc.scalar.activation(out=rstd, in_=mv[:, 1:2],
                                             func=AF.Sqrt, bias=epsS,
                                             scale=1.0)
                        nc.vector.reciprocal(rstd, rstd)
                        nc.vector.tensor_scalar(
                            out=oS, in0=r2s[mt], scalar1=mv[:, 0:1],
                            scalar2=rstd, op0=ALU.subtract, op1=ALU.mult)
                        if not cfg.ln2_trivial:
                            nc.vector.tensor_mul(oS, oS, g2S)
                            nc.vector.tensor_add(oS, oS, b2S)
                    else:
                        layer_norm(nc, pmd, oS, r2s[mt], g2S, b2S, epsS,
                                   "ln2",
                                   rows=False if cfg.ln2_trivial
                                   else nc.vector)
                    eng = [nc.scalar, nc.sync][i % 2] if tail else nc.scalar
                    eng.dma_start(out=out[mt * P:(mt + 1) * P, :], in_=oS)

            if fc_inter:
                hTc0 = pd.tile([P, MT_FF, TC], BF, tag="hTc", bufs=2,
                               name="hTc0")
                # fc chunk 0 mostly ran inside attention: batched gelu here
                for m in range(NFC_I):
                    nc.scalar.activation(
                        out=hTc0[:, m, :], in_=preact[:, m, :],
                        func=AF.Gelu_apprx_tanh, scale=1.0)
                for m in range(NFC_I, MT_FF):
                    fc_mtile_d(m, 0, hTc0)
                hTc1 = pd.tile([P, MT_FF, TC], BF, tag="hTc", bufs=2,
                               name="hTc1")
                # 2:1 interleave — 64 mproj kt-units vs 32 fc m-tiles
                mleft = MT_FF
                k = 0
                for _ in mproj_chunk(0, hTc0):
                    k += 1
                    if k % 2 == 0 and mleft > 0:
                        fc_mtile_d(MT_FF - mleft, 1, hTc1)
                        mleft -= 1
                while mleft > 0:
                    fc_mtile_d(MT_FF - mleft, 1, hTc1)
                    mleft -= 1
                ln2_out(list(range(TC // P)))  # overlaps mproj chunk 1
                for _ in mproj_chunk(TC, hTc1, split_stats=True):
                    pass
                ln2_out(list(range(TC // P, NQ_T)), tail=True)
            else:
                for t0, tw in chunks(NQ, TC):
                    hTc = pd.tile([P, MT_FF, TC], BF, tag="hTc", bufs=2)
                    for m in range(MT_FF):
                        fc_mtile_d(m, t0 // 512, hTc)
                    for _ in mproj_chunk(t0, hTc):
                        pass
                    ln2_out(list(range(t0 // P, (t0 + tw) // P)))

    nc.finalize()
    return nc


def host_inputs(cfg: Cfg, x, w_attn, b_attn, w_aproj, b_aproj, g1, b1,
                w_fc, b_fc, w_mproj, b_mproj, g2, b2, n_cores=8):
    """Build the per-core input maps + output scatter indices."""
    S, D = cfg.S, cfg.D
    NSLOT, KT, MT_FF = cfg.NSLOT, cfg.KT, cfg.MT_FF

    DK = cfg.DK
    wa = np.asarray(w_attn, np.float32)
    # q,k columns -> [p, col-tile, dk, j, c]; contraction row 256dk+128j+p
    wqk = np.ascontiguousarray(
        (wa[:, :2 * D] * WSCL).reshape(DK, 2, P, 2 * KT, P)
        .transpose(2, 3, 0, 1, 4)).astype(E4M3)
    # v columns -> [p, ci, dk, j, c] with 512-wide col chunks
    wv = np.ascontiguousarray(
        (wa[:, 2 * D:] * WSCL).reshape(DK, 2, P, 2, 512)
        .transpose(2, 3, 0, 1, 4)).astype(E4M3)
    b_adj = np.asarray(b_attn, np.float32).copy()
    b_adj[:D] *= 0.125
    bqkv = np.ascontiguousarray(b_adj[:2 * D].reshape(2 * KT, P).T, np.float32)
    shared = dict(
        wqk=wqk,
        wv=wv,
        bqkv=bqkv,
        bvrow=np.ascontiguousarray(
            np.asarray(b_attn, np.float32)[2 * D:].reshape(1, D)
        ).astype(BF16),
        wap=np.ascontiguousarray(
            (np.asarray(w_aproj, np.float32) * WSCL).reshape(DK, 2, P, D)
            .transpose(2, 0, 1, 3)).astype(E4M3),
        wfc=np.ascontiguousarray(
            (np.asarray(w_fc, np.float32)
             * np.asarray(g1, np.float32)[:, None])
            .reshape(KT, P, MT_FF, P).transpose(1, 2, 0, 3)).astype(BF16),
        bfc=np.ascontiguousarray(
            (np.asarray(b_fc, np.float32)
             + np.asarray(b1, np.float32) @ np.asarray(w_fc, np.float32))
            .reshape(MT_FF, P).T),
        wmp=np.asarray(w_mproj).astype(BF16),
        bmp=(np.asarray(b_mproj, np.float32)
             + np.asarray(b1, np.float32)).reshape(1, D),
        g1=np.asarray(g1, np.float32).reshape(1, D),
        g2=np.asarray(g2, np.float32).reshape(1, D),
        b2=np.asarray(b2, np.float32).reshape(1, D),
    )

    tri_kq = (np.arange(P)[:, None] <= np.arange(P)[None, :]).astype(np.float32)
    ones = np.ones((P, P), np.float32)
    zeros = np.zeros((P, P), np.float32)

    def m4(r):
        if r == 0:
            rows = [[tri_kq, ones], [zeros, ones], [zeros, tri_kq],
                    [zeros, zeros]]
        else:
            rows = [[ones, ones], [tri_kq, ones], [zeros, ones],
                    [zeros, tri_kq]]
        return np.stack([np.concatenate(rr, axis=1) for rr in rows])

    m_even, m_odd = m4(0), m4(1)

    in_maps = []
    idx_all = []
    for c in range(n_cores):
        b, r = c // 2, c % 2
        idx = (np.arange(NSLOT)[:, None] * 256 + 128 * r
               + np.arange(P)[None, :]).ravel()
        idx_all.append((b, idx))
        xb = np.asarray(x[b], np.float32)
        m = dict(shared)
        m["xt"] = np.ascontiguousarray(
            xb.T.reshape(DK, 2, P, S).transpose(2, 0, 1, 3)).astype(E4M3)
        m["xtq"] = np.ascontiguousarray(
            xb[idx].T.reshape(DK, 2, P, len(idx))
            .transpose(2, 0, 1, 3)).astype(E4M3)
        # fold the aproj bias into the residual base
        m["xq"] = np.ascontiguousarray(
            xb[idx] + np.asarray(b_aproj, np.float32)[None, :], np.float32)
        m["masks"] = (m_even if r == 0 else m_odd).astype(BF16)
        in_maps.append(m)
    return in_maps, idx_all


_CACHE = {}


def _get_nc(cfg: Cfg):
    key = (cfg.S, cfg.D, cfg.H, cfg.DFF, cfg.ln2_trivial, cfg.ln1_trivial)
    if key not in _CACHE:
        _CACHE[key] = build_nc(cfg)
    return _CACHE[key]


def kernel(x, w_attn, b_attn, w_aproj, b_aproj, g1, b1, w_fc, b_fc,
           w_mproj, b_mproj, g2, b2):
    trivial = bool(np.all(np.asarray(g2) == 1.0)
                   and np.all(np.asarray(b2) == 0.0))
    ln1_triv = bool(np.all(np.asarray(g1) == 1.0)
                    and np.all(np.asarray(b1) + np.asarray(b_mproj) == 0.0))
    cfg = Cfg(ln2_trivial=trivial, ln1_trivial=ln1_triv)
    nc = _get_nc(cfg)
    in_maps, idx_all = host_inputs(cfg, x, w_attn, b_attn, w_aproj, b_aproj,
                                   g1, b1, w_fc, b_fc, w_mproj, b_mproj,
                                   g2, b2)
    res = run_bass_kernel_spmd(nc, in_maps, list(range(8)))
    B = x.shape[0]
    y = np.empty((B, cfg.S, cfg.D), np.float32)
    for c in range(8):
        b, idx = idx_all[c]
        y[b][idx] = np.asarray(res.results[c]["out"]).astype(np.float32)
    return y



# revision 81
# speedup vs baseline: 1.0509x; 1.0509x over previous
"""Trainium2 Bass kernel for a GPT-2 style transformer block, 8-core SPMD.

Sharding: core c handles sequence c//2, query-parity c%2. Each core owns the
8 interleaved 128-token query blocks (even core: blocks 0,2,..,14; odd core:
blocks 1,3,..,15) of its sequence, computes full k/v for that sequence, runs
causal attention for its query blocks (all 16 heads), then the MLP for its
1024 tokens. Outputs are scattered back on the host. No collectives; the
per-core programs are instruction-identical (SPMD) — causality differences
are encoded in per-core mask *data* (multiplicative {0,1} masks on the last
two key-tiles of each query block).

Attention layout: scores are computed transposed (sT[key, q] = k . q) so that
both PV operands are keys-on-partition and softmax needs no transposes; the
denominator comes from an appended ones-column in V (out = [av | sum(p)]),
and the per-row normalization happens token-major where it is a cheap
per-partition scalar multiply.

Precision: qkv + aproj run fp8-e4m3 DoubleRow (weights pre-scaled x64,
undone at PSUM readout; 2 contraction rows per partition -> 256-deep
matmuls); attention core and the MLP stay bf16 (fp8 MLP exceeds the error
budget). Residual/layernorm fp32.

Scheduling: attention, aproj and LN1 are fused in one g-outer/head-inner
phase; PV is staggered one head behind scores so the PE streams while
ScalarE runs exp. The late k/v projection halves and most of the fc
matmuls are interleaved into the attention head slots — this feeds the PE
fat N=512 matmuls throughout, which keeps the HAM clock-gate at 2.4 GHz
(the small score/PV matmuls alone leave it throttled at 1.2 GHz) and
overlaps MLP work with the exp-bound softmax. fc's gelu is deferred and
batched so ScalarE never thrashes activation-table loads mid-phase.
"""

import sys
from contextlib import ExitStack

import numpy as np

for _p in ("/opt/trn_rl_repo",):
    if _p not in sys.path:
        sys.path.insert(0, _p)

import ml_dtypes

import concourse.bass as bass
import concourse.mybir as mybir
import concourse.tile as tile
from concourse import bacc
from concourse.bass_utils import run_bass_kernel_spmd
from concourse.masks import make_identity

BF16 = ml_dtypes.bfloat16
E4M3 = ml_dtypes.float8_e4m3
F32 = mybir.dt.float32
BF = mybir.dt.bfloat16
F8 = mybir.dt.float8e4
P = 128
AF = mybir.ActivationFunctionType
ALU = mybir.AluOpType
DR = mybir.MatmulPerfMode.DoubleRow
WSCL = 64.0  # fp8 weight pre-scale (std 0.02 -> 1.28), undone at PSUM readout


def chunks(total, size=512):
    return [(s, min(size, total - s)) for s in range(0, total, size)]


class Cfg:
    """Problem geometry. Defaults = the real problem; overridable for sims."""

    def __init__(self, S=2048, D=1024, H=16, DFF=4096, ln2_trivial=False,
                 ln1_trivial=False):
        self.ln2_trivial = ln2_trivial
        self.ln1_trivial = ln1_trivial
        self.S = S
        self.D = D
        self.H = H
        self.DFF = DFF
        self.HD = 64
        assert self.H * self.HD == self.D
        self.NQ = S // 2
        self.NSLOT = S // 256
        self.KT = D // P
        self.DK = D // (2 * P)
        self.MT_FF = DFF // P
        self.NQ_T = self.NQ // P
        self.S_T = S // P


def build_nc(cfg: Cfg):
    S, D, H, DFF = cfg.S, cfg.D, cfg.H, cfg.DFF
    NQ, NSLOT, KT, MT_FF = cfg.NQ, cfg.NSLOT, cfg.KT, cfg.MT_FF
    NQ_T, S_T, DK = cfg.NQ_T, cfg.S_T, cfg.DK
    NPAIR = H // 2

    nc = bacc.Bacc(None, target_bir_lowering=False, debug=False)

    def din(name, shape, d=BF):
        return nc.dram_tensor(name, shape, d, kind="ExternalInput").ap()

    # fp8 DoubleRow operands: contraction row 256*dk + 128*j + p lives at
    # [p, dk, j] so a [P, 2, cols] slice contracts 256 rows per matmul.
    xt = din("xt", [P, DK, 2, S], F8)      # x[b].T packed for DoubleRow
    xtq = din("xtq", [P, DK, 2, NQ], F8)   # own tokens, same packing
    xq = din("xq", [NQ, D], F32)           # own tokens, token-major (residual)
    masks = din("masks", [4, P, 2 * P])    # multiplicative causal masks (bf16)
    wqk = din("wqk", [P, 2 * KT, DK, 2, P], F8)   # q,k col-tiles (x WSCL)
    wv = din("wv", [P, 2, DK, 2, 512], F8)        # v col-chunks (x WSCL)
    bqkv = din("bqkv", [P, 2 * KT], F32)   # q,k bias packed (col%128, col//128)
    bvrow = din("bvrow", [1, D])           # v bias as a row (bf16)
    wap = din("wap", [P, DK, 2, D], F8)    # aproj rows (x WSCL)
    wfc = din("wfc", [P, MT_FF, KT, P])
    bfc = din("bfc", [P, MT_FF], F32)
    wmp = din("wmp", [DFF, D])
    bmp = din("bmp", [1, D], F32)
    g1 = din("g1", [1, D], F32)
    g2 = din("g2", [1, D], F32)
    b2 = din("b2", [1, D], F32)
    # bf16 output halves the writeback (the final tail); host upcasts.
    # Costs ~0.11% rms on top of 2.8e-3 total err vs the 2e-2 gate.
    out = nc.dram_tensor("out", [NQ, D], BF, kind="ExternalOutput").ap()

    wmp3 = wmp.rearrange("(kt p) c -> kt p c", p=P)

    def bcast_row(ap):  # [1, D] DRAM -> [P, D] broadcast AP
        return bass.AP(tensor=ap.tensor, offset=ap.offset, ap=[[0, P], [1, D]])

    NSUB = (D + 511) // 512
    SUB = D // NSUB
    assert SUB * NSUB == D and SUB <= 512

    def layer_norm(nc, pool, dst, src, g_row, b_row, eps_t, tag, rows=None):
        """dst[P, D] = g * (src - mean)/sqrt(var+eps) + b, rowwise over D.
        rows=False skips the g/b application entirely."""
        skip_rows = rows is False
        rows = nc.gpsimd if rows in (None, False) else rows
        stats = pool.tile([P, NSUB, 6], F32, tag=f"{tag}_st")
        for sub in range(NSUB):
            nc.vector.bn_stats(stats[:, sub, :], src[:, sub * SUB:(sub + 1) * SUB])
        mv = pool.tile([P, 2], F32, tag=f"{tag}_mv")
        nc.vector.bn_aggr(mv, stats)
        rstd = pool.tile([P, 1], F32, tag=f"{tag}_rs")
        nc.scalar.activation(out=rstd, in_=mv[:, 1:2], func=AF.Sqrt,
                             bias=eps_t, scale=1.0)
        nc.vector.reciprocal(rstd, rstd)
        nc.vector.tensor_scalar(
            out=dst, in0=src, scalar1=mv[:, 0:1], scalar2=rstd,
            op0=ALU.subtract, op1=ALU.mult)
        if not skip_rows:
            rows.tensor_mul(dst, dst, g_row)
            rows.tensor_add(dst, dst, b_row)

    with tile.TileContext(nc) as tc, ExitStack() as top:
        const = top.enter_context(tc.tile_pool(name="const", bufs=1))
        bqkvS = const.tile([P, 2 * KT], F32)
        nc.sync.dma_start(out=bqkvS, in_=bqkv)
        epsS = const.tile([P, 1], F32)
        nc.vector.memset(epsS, 1e-5)
        bvS = const.tile([P, D], BF)  # dma issued later, off the q critical path

        npool = top.enter_context(tc.tile_pool(name="npool", bufs=1))
        n_sb = [npool.tile([P, D], BF, name=f"n{mt}", tag=f"n{mt}")
                for mt in range(NQ_T)]
        NHALF = max(1, NQ // 512)
        nT = [[npool.tile([P, min(512, NQ)], BF, name=f"nT{ck}_{hf}",
                          tag=f"nT{ck}_{hf}") for hf in range(NHALF)]
              for ck in range(KT)]
        # fc token-chunk 0 is interleaved into the attention phase (keeps the
        # PE HAM-warm through the small-matmul stream); gelu is deferred to
        # phase D so ScalarE never switches tables away from Exp mid-phase.
        fc_inter = (MT_FF == 2 * H and NQ >= 512 and NHALF == 2)
        NFC_I = MT_FF - 5 if fc_inter else 0  # m-tiles run inside attention
        if fc_inter:
            preact = npool.tile([P, NFC_I, 512], BF)
            bfcSe = npool.tile([P, MT_FF], F32)

        with ExitStack() as s_a:
            preh = s_a.enter_context(tc.tile_pool(name="pre_xt", bufs=1))
            aper = s_a.enter_context(tc.tile_pool(name="aper", bufs=1))
            maskS = aper.tile([P, 4, 2 * P], BF)

            with ExitStack() as s_qkv:
                qkvp = s_qkv.enter_context(tc.tile_pool(name="qkvper", bufs=1))
                qT = qkvp.tile([P, NPAIR, NQ], BF)
                kT = qkvp.tile([P, NPAIR, S], BF)
                vA = qkvp.tile([P, S_T, H, 65], BF)

                # ---------------- Phase A: qkv projections (fp8) ----------
                # Startup transfers: each trigger engine's DMAs serialize on
                # its queue, so the q critical path (xtq + wq) issues FIRST
                # on all three queues; bulk/late loads (xtS, masks, biases)
                # queue up behind them.
                xtS = preh.tile([P, DK, 2, S], F8, tag="bigslot")

                # A1: q projection (own tokens)
                with ExitStack() as ph:
                    pa = ph.enter_context(tc.tile_pool(name="pa1", bufs=1))
                    wp = ph.enter_context(tc.tile_pool(name="pa1_w", bufs=3))
                    pp = ph.enter_context(
                        tc.tile_pool(name="pa1_ps", bufs=4, space="PSUM"))
                    xtqS = pa.tile([P, DK, 2, NQ], F8)
                    # first token-half split by dk across two queues so the
                    # first q matmuls unblock in ~half the transfer time
                    nc.sync.dma_start(out=xtqS[:, 0:DK // 2, :, 0:NQ // 2],
                                      in_=xtq[:, 0:DK // 2, :, 0:NQ // 2])
                    nc.gpsimd.dma_start(out=xtqS[:, DK // 2:, :, 0:NQ // 2],
                                        in_=xtq[:, DK // 2:, :, 0:NQ // 2])
                    wqa = wp.tile([P, NPAIR, DK, 2, P], F8, tag="wq", bufs=1)
                    nc.scalar.dma_start(out=wqa[:, 0:NPAIR // 2],
                                        in_=wqk[:, 0:NPAIR // 2])
                    nc.gpsimd.dma_start(out=wqa[:, NPAIR // 2:NPAIR],
                                        in_=wqk[:, NPAIR // 2:NPAIR])
                    nc.sync.dma_start(out=xtqS[:, :, :, NQ // 2:],
                                      in_=xtq[:, :, :, NQ // 2:])
                    # bulk loads, behind the q-path on each queue
                    dma_engs = [nc.gpsimd, nc.scalar, nc.gpsimd, nc.sync]
                    for i, (h0, hw) in enumerate(chunks(S, S // 4)):
                        dma_engs[i].dma_start(
                            out=xtS[:, :, :, h0:h0 + hw],
                            in_=xt[:, :, :, h0:h0 + hw])
                    nc.sync.dma_start(
                        out=maskS, in_=masks.rearrange("m k q -> k m q"))
                    nc.scalar.dma_start(out=bvS, in_=bcast_row(bvrow))
                    if fc_inter:
                        nc.sync.dma_start(out=bfcSe, in_=bfc)
                    for c0, w in chunks(NQ):
                        for hp in range(NPAIR):
                            ps = pp.tile([P, 512], F32, tag="ps")
                            for dk in range(DK):
                                nc.tensor.matmul(
                                    ps[:, :w], lhsT=wqa[:, hp, dk],
                                    rhs=xtqS[:, dk, :, c0:c0 + w],
                                    start=(dk == 0), stop=(dk == DK - 1),
                                    perf_mode=DR)
                            nc.scalar.activation(
                                out=qT[:, hp, c0:c0 + w], in_=ps[:, :w],
                                func=AF.Identity, bias=bqkvS[:, hp:hp + 1],
                                scale=1.0 / (8.0 * WSCL))

                # A2: k/v projections. The late halves (k columns >= S/2,
                # v token-tiles >= S_T/2) are deferred into the g0/g1
                # attention slots: keeps the PE HAM-warm there and shortens
                # the serial phase A. Consumers only need them from g2 on.
                wvS = qkvp.tile([P, 2, DK, 2, 512], F8)
                nc.gpsimd.dma_start(out=wvS, in_=wv)

                def k_unit(wk, hp, c0, pstile, eng):
                    w = 512
                    for dk in range(DK):
                        nc.tensor.matmul(
                            pstile[:, :w], lhsT=wk[:, dk],
                            rhs=xtS[:, dk, :, c0:c0 + w],
                            start=(dk == 0), stop=(dk == DK - 1),
                            perf_mode=DR)
                    eng.tensor_scalar(
                        out=kT[:, hp, c0:c0 + w], in0=pstile[:, :w],
                        scalar1=1.0 / WSCL,
                        scalar2=bqkvS[:, KT + hp:KT + hp + 1],
                        op0=ALU.mult, op1=ALU.add)

                def v_unit(ci, mt, pstile):
                    c0, cw = ci * 512, 512
                    h0, nh = c0 // 64, cw // 64
                    for dk in range(DK):
                        nc.tensor.matmul(
                            pstile[:, :cw],
                            lhsT=xtS[:, dk, :, mt * P:(mt + 1) * P],
                            rhs=wvS[:, ci, dk],
                            start=(dk == 0), stop=(dk == DK - 1),
                            perf_mode=DR)
                    nc.vector.scalar_tensor_tensor(
                        out=vA[:, mt, h0:h0 + nh, 0:64],
                        in0=pstile[:, :cw]
                        .rearrange("p (h d) -> p h d", d=64),
                        scalar=1.0 / WSCL,
                        in1=bvS[:, c0:c0 + cw]
                        .rearrange("p (h d) -> p h d", d=64),
                        op0=ALU.mult, op1=ALU.add)

                kv_defer = (S == 2048 and NPAIR == 8 and S_T == 16)
                with ExitStack() as ph:
                    wp = ph.enter_context(tc.tile_pool(name="pa2_w", bufs=3))
                    pp = ph.enter_context(
                        tc.tile_pool(name="pa2_ps", bufs=4, space="PSUM"))
                    kc = [(hp, c0) for hp in range(NPAIR)
                          for c0, _ in chunks(S)]
                    if kv_defer:
                        kc = [(hp, c0) for hp, c0 in kc if c0 < S // 2]
                    lasthp = None
                    for hp, c0 in kc:
                        if hp != lasthp:
                            wk = wp.tile([P, DK, 2, P], F8, tag="wk")
                            nc.scalar.dma_start(out=wk, in_=wqk[:, KT + hp])
                            lasthp = hp
                        k_unit(wk, hp, c0,
                               pp.tile([P, 512], F32, tag="ps", name="ps"),
                               nc.vector)
                    vc = [(ci, mt) for ci in range(2) for mt in range(S_T)]
                    if kv_defer:
                        vc = [(ci, mt) for ci, mt in vc if mt < S_T // 2]
                    for ci, mt in vc:
                        v_unit(ci, mt,
                               pp.tile([P, 512], F32, tag="ps", name="ps"))
                    nc.vector.memset(vA[:, :, :, 64:65], 1.0)

                # deferred-unit issue list for the g0/g1 attention slots:
                # k late chunks first (needed at g2), then v late tiles
                # v first: wvS is already resident (no per-unit weight DMA),
                # so g0 gets fat matmuls immediately; k units at g1 give the
                # weight prefetch a full group of slack. k needed from g2 on.
                kv_units = []
                if kv_defer:
                    for mt in range(S_T // 2, S_T):
                        for ci in range(2):
                            kv_units.append(("v", ci, mt))
                    for hp in range(NPAIR):
                        for c0 in (S // 2, S // 2 + 512):
                            kv_units.append(("k", hp, c0))

                # wap for the per-group aproj inside the fused phase
                wapS = preh.tile([P, DK, 2, D], F8, tag="wapslot",
                                 name="wapS")
                nc.gpsimd.dma_start(out=wapS, in_=wap)

                # ------- Phase B+C fused: attention + aproj + LN1 -------
                # g-outer/head-inner; PV staggered one head behind scores so
                # the PE streams scores(h+1) while ScalarE exps head h; aproj
                # + LN1 run per 256-token group to keep the PE warm (HAM).
                with ExitStack() as ph:
                    psp = ph.enter_context(
                        tc.tile_pool(name="pb_s", bufs=2, space="PSUM"))
                    pvp = ph.enter_context(
                        tc.tile_pool(name="pb_av", bufs=1, space="PSUM"))
                    paux = ph.enter_context(
                        tc.tile_pool(name="pb_aux", bufs=2, space="PSUM"))
                    ptp = ph.enter_context(tc.tile_pool(name="pb_pt", bufs=4))
                    pm = ph.enter_context(tc.tile_pool(name="pb_m", bufs=4))
                    pc = ph.enter_context(tc.tile_pool(name="pb_c", bufs=1))
                    pwf = ph.enter_context(tc.tile_pool(name="pb_wf", bufs=2))

                    idb = pc.tile([P, P], BF)
                    make_identity(nc, idb)

                    mt_per_half = NQ_T // NHALF

                    def head_scores(g, h, nkt, ch):
                        # ch: k-tiles per score chunk. Finer chunks in the
                        # early groups pipeline exp against the PE tighter
                        # (the PE waits on exp to free the PSUM ring).
                        hp, hh = h // 2, h % 2
                        pb = hh * 64
                        pts = []
                        for sc in range(nkt // ch):
                            lo = sc * ch
                            ps = psp.tile([P, ch, 2 * P], F32, tag="ps",
                                          padded_shape=[P, 4, 2 * P])
                            for kt in range(lo, lo + ch):
                                nc.tensor.matmul(
                                    ps[:, kt - lo, :],
                                    lhsT=kT[pb:pb + 64, hp,
                                            kt * P:(kt + 1) * P],
                                    rhs=qT[pb:pb + 64, hp,
                                           g * 2 * P:(g + 1) * 2 * P],
                                    start=True, stop=True)
                            pt = ptp.tile([P, ch, 2 * P], BF,
                                          tag=f"pt{h % 2}", name="pt",
                                          padded_shape=[P, 4, 2 * P])
                            nc.scalar.activation(out=pt, in_=ps, func=AF.Exp)
                            if lo >= nkt - 4:  # masks cover last 4 k-tiles
                                mo = lo - (nkt - 4)
                                nc.vector.tensor_mul(
                                    pt, pt, maskS[:, mo:mo + ch])
                            pts.append(pt)
                        return pts

                    def head_pv(g, h, nkt, ch, pts, aL, aR):
                        pavL = pvp.tile([P, 65], F32, tag="pavL")
                        pavR = pvp.tile([P, 65], F32, tag="pavR")
                        for kt in range(nkt - 2):
                            nc.tensor.matmul(
                                pavL, lhsT=pts[kt // ch][:, kt % ch, 0:P],
                                rhs=vA[:, kt, h, :],
                                start=(kt == 0), stop=(kt == nkt - 3))
                        for kt in range(nkt):
                            nc.tensor.matmul(
                                pavR, lhsT=pts[kt // ch][:, kt % ch,
                                                         P:2 * P],
                                rhs=vA[:, kt, h, :],
                                start=(kt == 0), stop=(kt == nkt - 1))
                        for pav, aB in ((pavL, aL), (pavR, aR)):
                            rec = pm.tile([P, 1], F32, tag="rec")
                            nc.vector.reciprocal(rec, pav[:, 64:65])
                            nc.vector.tensor_scalar_mul(
                                out=aB[:, h * 64:(h + 1) * 64],
                                in0=pav[:, 0:64], scalar1=rec)

                    def ntr(mt):  # n transpose -> nT (fc operand)
                        hf, mo = mt // mt_per_half, mt % mt_per_half
                        pstf = paux.tile([P, KT, P], BF, tag="paux",
                                         name="pstf")
                        for ck in range(KT):
                            nc.tensor.transpose(
                                pstf[:, ck], n_sb[mt][:, ck * P:(ck + 1) * P],
                                idb)
                        for ck in range(KT):
                            nc.vector.tensor_scalar_mul(
                                out=nT[ck][hf][:, mo * P:(mo + 1) * P],
                                in0=pstf[:, ck], scalar1=1.0)

                    kv_wk = {}

                    def kv_prefetch(ui):
                        # prefetch the k-pair weight for units [ui, ui+1]
                        if ui % 2 or ui >= len(kv_units) or ui in kv_wk:
                            return
                        kind, a, _ = kv_units[ui]
                        if kind != "k":
                            return
                        wkd = pwf.tile([P, DK, 2, P], F8, tag="wfc",
                                       name="wkd")
                        (nc.gpsimd if ui % 4 else nc.sync).dma_start(
                            out=wkd, in_=wqk[:, KT + a])
                        kv_wk[ui] = wkd

                    def kv_slot(ui):
                        kind, a, b = kv_units[ui]
                        pstile = paux.tile([P, 512], F32, tag="paux",
                                           name="kvps")
                        if kind == "k":
                            base = ui - (ui % 2)
                            kv_prefetch(base)       # no-op if prefetched
                            kv_prefetch(base + 2)   # hide next pair's DMA
                            k_unit(kv_wk[base], a, b, pstile, nc.vector)
                            if ui % 2 == 1:
                                kv_wk.pop(base)
                        else:
                            v_unit(a, b, pstile)
                            if ui + 2 < len(kv_units) \
                                    and kv_units[ui + 2][0] == "k":
                                kv_prefetch(ui + 2)

                    fc_w = {}

                    def fc_prefetch(m):
                        if m >= NFC_I or m in fc_w:
                            return
                        wfcT = pwf.tile([P, KT, P], BF, tag="wfc",
                                        name="wfcT")
                        (nc.gpsimd if m % 2 else nc.sync).dma_start(
                            out=wfcT, in_=wfc[:, m])
                        fc_w[m] = wfcT

                    def fc_mtile(m):
                        # weight was prefetched one slot ahead — the DMA
                        # latency is hidden behind the previous slot's work
                        wfcT = fc_w.pop(m)
                        fcps = paux.tile([P, 512], F32, tag="paux",
                                         name="fcps")
                        for kt in range(KT):
                            nc.tensor.matmul(
                                fcps, lhsT=wfcT[:, kt, :],
                                rhs=nT[kt][0][:, 0:512],
                                start=(kt == 0), stop=(kt == KT - 1))
                        nc.vector.tensor_scalar_add(
                            out=preact[:, m], in0=fcps,
                            scalar1=bfcSe[:, m:m + 1])

                    def block_post(j, aB, jprev):
                        # a^T (fp8 DoubleRow operand) via PE transpose
                        pst = paux.tile([P, KT, P], BF, tag="paux",
                                        name="pst")
                        for ck in range(KT):
                            nc.tensor.transpose(
                                pst[:, ck], aB[:, ck * P:(ck + 1) * P], idb)
                        aTb = [pc.tile([P, 2, P], F8, name=f"aT{dk}",
                                       tag=f"aT{dk}", bufs=1)
                               for dk in range(DK)]
                        for ck in range(KT):
                            nc.vector.tensor_scalar_mul(
                                out=aTb[ck // 2][:, ck % 2, :],
                                in0=pst[:, ck], scalar1=1.0)
                        xqS = pm.tile([P, D], F32, tag="xqS", bufs=1)
                        nc.sync.dma_start(out=xqS,
                                          in_=xq[j * P:(j + 1) * P, :])
                        for c0, w in chunks(D):
                            aps = paux.tile([P, 512], F32, tag="paux",
                                            name="aps")
                            for dk in range(DK):
                                nc.tensor.matmul(
                                    aps[:, :w],
                                    lhsT=aTb[dk],
                                    rhs=wapS[:, dk, :, c0:c0 + w],
                                    start=(dk == 0), stop=(dk == DK - 1),
                                    perf_mode=DR)
                            sl = slice(c0, c0 + w)
                            nc.vector.scalar_tensor_tensor(
                                out=xqS[:, sl], in0=aps[:, :w],
                                scalar=1.0 / WSCL,
                                in1=xqS[:, sl], op0=ALU.mult, op1=ALU.add)
                        layer_norm(nc, pm, n_sb[j], xqS, None, None,
                                   epsS, "ln1", rows=False)
                        if jprev is not None:
                            ntr(jprev)  # PE work overlapping LN1(j) on DVE

                    prev = None
                    jprev = None
                    NG = NSLOT // 2
                    for g in range(NG):
                        nkt = 4 * g + 4
                        jL, jR = 2 * g, 2 * g + 1
                        aL = aper.tile([P, D], BF, tag="aSB", bufs=3,
                                       name=f"aSB{jL}")
                        aR = aper.tile([P, D], BF, tag="aSB", bufs=3,
                                       name=f"aSB{jR}")
                        for h in range(H):
                            ch = 4
                            pts = head_scores(g, h, nkt, ch)
                            if prev is not None:
                                head_pv(*prev)
                            prev = (g, h, nkt, ch, pts, aL, aR)
                            if fc_inter and g >= NG - 2:
                                mi = (g - (NG - 2)) * H + h
                                fc_prefetch(mi + 1)
                                if mi < NFC_I:
                                    fc_mtile(mi)
                            elif kv_units and g < 2:
                                ui = g * H + h
                                if ui < len(kv_units):
                                    kv_slot(ui)
                        head_pv(*prev)
                        prev = None
                        block_post(jL, aL, jprev)
                        block_post(jR, aR, jL)
                        jprev = jR
                        if fc_inter and g == NG - 3:
                            # flush: g+1 interleaves fc reads of nT[..][0],
                            # which needs block jR's transpose done now
                            ntr(jR)
                            jprev = None
                            fc_prefetch(0)  # first fc weight, one g early
                    ntr(jprev)
            # qT/kT/vA freed here

            # residual copy: n = g1*n0 + (b1 + b_mproj), off critical path
            if not cfg.ln1_trivial:
                g1S = npool.tile([P, D], F32)
                nc.sync.dma_start(out=g1S, in_=bcast_row(g1))
                bmpS = npool.tile([P, D], F32)
                nc.sync.dma_start(out=bmpS, in_=bcast_row(bmp))
                for mt in range(NQ_T):
                    nc.gpsimd.tensor_mul(n_sb[mt], n_sb[mt], g1S)
                    nc.gpsimd.tensor_add(n_sb[mt], n_sb[mt], bmpS)
        # aSB freed here

        # -------- Phase D: MLP + residual + LN2, in half-token chunks --------
        with ExitStack() as ph:
            pd = ph.enter_context(tc.tile_pool(name="pd", bufs=1))
            pdw = ph.enter_context(tc.tile_pool(name="pd_w", bufs=8))
            pmd = ph.enter_context(tc.tile_pool(name="pd_m", bufs=2))
            ppd = ph.enter_context(
                tc.tile_pool(name="pd_ps", bufs=2, space="PSUM"))
            ppm = ph.enter_context(
                tc.tile_pool(name="pd_psm", bufs=1, space="PSUM"))

            bfcS = pd.tile([P, MT_FF], F32)
            nc.sync.dma_start(out=bfcS, in_=bfc)
            g2S = pd.tile([P, D], F32)
            nc.sync.dma_start(out=g2S, in_=bcast_row(g2))
            b2S = pd.tile([P, D], F32)
            nc.sync.dma_start(out=b2S, in_=bcast_row(b2))

            TC = min(512, NQ)          # token-chunk

            def fc_mtile_d(m, hf, hTc):
                wfcT = pdw.tile([P, KT, P], BF, tag="wfc")
                nc.scalar.dma_start(out=wfcT, in_=wfc[:, m])
                ps = ppd.tile([P, 512], F32, tag="ps")
                for kt in range(KT):
                    nc.tensor.matmul(
                        ps, lhsT=wfcT[:, kt, :], rhs=nT[kt][hf][:, 0:TC],
                        start=(kt == 0), stop=(kt == KT - 1))
                nc.scalar.activation(
                    out=hTc[:, m, :], in_=ps,
                    func=AF.Gelu_apprx_tanh, bias=bfcS[:, m:m + 1],
                    scale=1.0)

            r2s = {}
            st2 = {}

            def mproj_chunk(t0, hTc, split_stats=False):
                # generator: yields after each kt unit so fc work for the
                # next chunk can interleave into the mproj weight stream.
                # split_stats: issue each row-block's first-half LN2 stats
                # right after the ci=0 residual, shortening the final tail.
                mts = list(range(t0 // P, (t0 + TC) // P))
                for mt in mts:
                    r2s[mt] = pmd.tile([P, D], F32, tag=f"r2_{mt % 4}",
                                       name=f"r2_{mt}")
                for ci, (c0, w) in enumerate(chunks(D)):
                    pss = {mt: ppm.tile([P, 512], F32, tag=f"mp{mt % 4}",
                                        name=f"mp_{mt}_{ci}")
                           for mt in mts}
                    for kt in range(MT_FF):
                        wmT = pdw.tile([P, 512], BF, tag="wm")
                        nc.sync.dma_start(out=wmT[:, :w],
                                          in_=wmp3[kt][:, c0:c0 + w])
                        for mt in mts:
                            nc.tensor.matmul(
                                pss[mt][:, :w],
                                lhsT=hTc[:, kt, mt * P - t0:(mt + 1) * P - t0],
                                rhs=wmT[:, :w],
                                start=(kt == 0), stop=(kt == MT_FF - 1))
                        yield
                    sl = slice(c0, c0 + w)
                    for mt in mts:
                        nc.vector.scalar_tensor_tensor(
                            out=r2s[mt][:, sl], in0=pss[mt][:, :w],
                            scalar=1.0, in1=n_sb[mt][:, sl],
                            op0=ALU.mult, op1=ALU.add)
                        if split_stats and NSUB == 2:
                            if ci == 0:
                                st2[mt] = pmd.tile([P, NSUB, 6], F32,
                                                   tag=f"st2_{mt % 4}",
                                                   name=f"st2_{mt}")
                            nc.vector.bn_stats(st2[mt][:, ci, :],
                                               r2s[mt][:, sl])

            def ln2_out(mts, tail=False):
                for i, mt in enumerate(mts):
                    oS = pmd.tile([P, D], BF, tag="oS")
                    if tail and mt in st2:
                        # stats were issued inside mproj_chunk; finish only
                        # aggr + rstd + normalize here
                        mv = pmd.tile([P, 2], F32, tag="ln2f_mv")
                        nc.vector.bn_aggr(mv, st2[mt])
                        rstd = pmd.tile([P, 1], F32, tag="ln2f_rs")
                        nc.scalar.activation(out=rstd, in_=mv[:, 1:2],
                                             func=AF.Sqrt, bias=epsS,
                                             scale=1.0)
                        nc.vector.reciprocal(rstd, rstd)
                        nc.vector.tensor_scalar(
                            out=oS, in0=r2s[mt], scalar1=mv[:, 0:1],
                            scalar2=rstd, op0=ALU.subtract, op1=ALU.mult)
                        if not cfg.ln2_trivial:
                            nc.vector.tensor_mul(oS, oS, g2S)
                            nc.vector.tensor_add(oS, oS, b2S)
                    else:
                        layer_norm(nc, pmd, oS, r2s[mt], g2S, b2S, epsS,
                                   "ln2",
                                   rows=False if cfg.ln2_trivial
                                   else nc.vector)
                    eng = [nc.scalar, nc.sync][i % 2] if tail else nc.scalar
                    eng.dma_start(out=out[mt * P:(mt + 1) * P, :], in_=oS)

            if fc_inter:
                hTc0 = pd.tile([P, MT_FF, TC], BF, tag="hTc", bufs=2,
                               name="hTc0")
                # fc chunk 0 mostly ran inside attention: batched gelu here
                for m in range(NFC_I):
                    nc.scalar.activation(
                        out=hTc0[:, m, :], in_=preact[:, m, :],
                        func=AF.Gelu_apprx_tanh, scale=1.0)
                for m in range(NFC_I, MT_FF):
                    fc_mtile_d(m, 0, hTc0)
                hTc1 = pd.tile([P, MT_FF, TC], BF, tag="hTc", bufs=2,
                               name="hTc1")
                # 2:1 interleave — 64 mproj kt-units vs 32 fc m-tiles
                mleft = MT_FF
                k = 0
                for _ in mproj_chunk(0, hTc0):
                    k += 1
                    if k % 2 == 0 and mleft > 0:
                        fc_mtile_d(MT_FF - mleft, 1, hTc1)
                        mleft -= 1
                while mleft > 0:
                    fc_mtile_d(MT_FF - mleft, 1, hTc1)
                    mleft -= 1
                ln2_out(list(range(TC // P)))  # overlaps mproj chunk 1
                for _ in mproj_chunk(TC, hTc1, split_stats=True):
                    pass
                ln2_out(list(range(TC // P, NQ_T)), tail=True)
            else:
                for t0, tw in chunks(NQ, TC):
                    hTc = pd.tile([P, MT_FF, TC], BF, tag="hTc", bufs=2)
                    for m in range(MT_FF):
                        fc_mtile_d(m, t0 // 512, hTc)
                    for _ in mproj_chunk(t0, hTc):
                        pass
                    ln2_out(list(range(t0 // P, (t0 + tw) // P)))

    nc.finalize()
    return nc


def host_inputs(cfg: Cfg, x, w_attn, b_attn, w_aproj, b_aproj, g1, b1,
                w_fc, b_fc, w_mproj, b_mproj, g2, b2, n_cores=8):
    """Build the per-core input maps + output scatter indices."""
    S, D = cfg.S, cfg.D
    NSLOT, KT, MT_FF = cfg.NSLOT, cfg.KT, cfg.MT_FF

    DK = cfg.DK
    wa = np.asarray(w_attn, np.float32)
    # q,k columns -> [p, col-tile, dk, j, c]; contraction row 256dk+128j+p
    wqk = np.ascontiguousarray(
        (wa[:, :2 * D] * WSCL).reshape(DK, 2, P, 2 * KT, P)
        .transpose(2, 3, 0, 1, 4)).astype(E4M3)
    # v columns -> [p, ci, dk, j, c] with 512-wide col chunks
    wv = np.ascontiguousarray(
        (wa[:, 2 * D:] * WSCL).reshape(DK, 2, P, 2, 512)
        .transpose(2, 3, 0, 1, 4)).astype(E4M3)
    b_adj = np.asarray(b_attn, np.float32).copy()
    b_adj[:D] *= 0.125
    bqkv = np.ascontiguousarray(b_adj[:2 * D].reshape(2 * KT, P).T, np.float32)
    shared = dict(
        wqk=wqk,
        wv=wv,
        bqkv=bqkv,
        bvrow=np.ascontiguousarray(
            np.asarray(b_attn, np.float32)[2 * D:].reshape(1, D)
        ).astype(BF16),
        wap=np.ascontiguousarray(
            (np.asarray(w_aproj, np.float32) * WSCL).reshape(DK, 2, P, D)
            .transpose(2, 0, 1, 3)).astype(E4M3),
        wfc=np.ascontiguousarray(
            (np.asarray(w_fc, np.float32)
             * np.asarray(g1, np.float32)[:, None])
            .reshape(KT, P, MT_FF, P).transpose(1, 2, 0, 3)).astype(BF16),
        bfc=np.ascontiguousarray(
            (np.asarray(b_fc, np.float32)
             + np.asarray(b1, np.float32) @ np.asarray(w_fc, np.float32))
            .reshape(MT_FF, P).T),
        wmp=np.asarray(w_mproj).astype(BF16),
        bmp=(np.asarray(b_mproj, np.float32)
             + np.asarray(b1, np.float32)).reshape(1, D),
        g1=np.asarray(g1, np.float32).reshape(1, D),
        g2=np.asarray(g2, np.float32).reshape(1, D),
        b2=np.asarray(b2, np.float32).reshape(1, D),
    )

    tri_kq = (np.arange(P)[:, None] <= np.arange(P)[None, :]).astype(np.float32)
    ones = np.ones((P, P), np.float32)
    zeros = np.zeros((P, P), np.float32)

    def m4(r):
        if r == 0:
            rows = [[tri_kq, ones], [zeros, ones], [zeros, tri_kq],
                    [zeros, zeros]]
        else:
            rows = [[ones, ones], [tri_kq, ones], [zeros, ones],
                    [zeros, tri_kq]]
        return np.stack([np.concatenate(rr, axis=1) for rr in rows])

    m_even, m_odd = m4(0), m4(1)

    in_maps = []
    idx_all = []
    for c in range(n_cores):
        b, r = c // 2, c % 2
        idx = (np.arange(NSLOT)[:, None] * 256 + 128 * r
               + np.arange(P)[None, :]).ravel()
        idx_all.append((b, idx))
        xb = np.asarray(x[b], np.float32)
        m = dict(shared)
        m["xt"] = np.ascontiguousarray(
            xb.T.reshape(DK, 2, P, S).transpose(2, 0, 1, 3)).astype(E4M3)
        m["xtq"] = np.ascontiguousarray(
            xb[idx].T.reshape(DK, 2, P, len(idx))
            .transpose(2, 0, 1, 3)).astype(E4M3)
        # fold the aproj bias into the residual base
        m["xq"] = np.ascontiguousarray(
            xb[idx] + np.asarray(b_aproj, np.float32)[None, :], np.float32)
        m["masks"] = (m_even if r == 0 else m_odd).astype(BF16)
        in_maps.append(m)
    return in_maps, idx_all


_CACHE = {}


def _get_nc(cfg: Cfg):
    key = (cfg.S, cfg.D, cfg.H, cfg.DFF, cfg.ln2_trivial, cfg.ln1_trivial)
    if key not in _CACHE:
        _CACHE[key] = build_nc(cfg)
    return _CACHE[key]


def kernel(x, w_attn, b_attn, w_aproj, b_aproj, g1, b1, w_fc, b_fc,
           w_mproj, b_mproj, g2, b2):
    trivial = bool(np.all(np.asarray(g2) == 1.0)
                   and np.all(np.asarray(b2) == 0.0))
    ln1_triv = bool(np.all(np.asarray(g1) == 1.0)
                    and np.all(np.asarray(b1) + np.asarray(b_mproj) == 0.0))
    cfg = Cfg(ln2_trivial=trivial, ln1_trivial=ln1_triv)
    nc = _get_nc(cfg)
    in_maps, idx_all = host_inputs(cfg, x, w_attn, b_attn, w_aproj, b_aproj,
                                   g1, b1, w_fc, b_fc, w_mproj, b_mproj,
                                   g2, b2)
    res = run_bass_kernel_spmd(nc, in_maps, list(range(8)))
    B = x.shape[0]
    y = np.empty((B, cfg.S, cfg.D), np.float32)
    for c in range(8):
        b, idx = idx_all[c]
        y[b][idx] = np.asarray(res.results[c]["out"]).astype(np.float32)
    return y

